# revision 4
# baseline (speedup 1.0000x reference)
"""GRU (ragged sequences) Trainium2 Bass kernel.

Data-parallel over batch: 8 cores x 8 sequences; weights replicated.
Per core the T-step recurrence runs as one lockstep chain in
[H=128 partitions, B=8 free] layout. The chain is latency-bound
(T strictly serial steps), so everything targets per-tick latency.

Measured HW facts this design is built around (micro-benchmarked):
  - an on-chain PE stationary-weight switch costs ~500ns (no FWL in
    this walrus path); an MM reusing the loaded stationary is ~50ns.
  - cross-engine sem hops are cheap (~30-60ns); ACT sigmoid ~150ns.
  - the PE queue is strict FIFO: a matmul parked on a semaphore blocks
    everything emitted after it.

Structure per tick t (h_{t-1} carried as the pair (mm, nsm) with
h = mm + nsm so the PE consumes both directly and the final blend
leaves the serial chain; W@h = W@mm + W@nsm uses ONE stationary per
gate):

  pre-mm (early operands only, pays all stationary switches off-chain):
    ps_r  = id @ gi_r[t]                 (start)
    ps_u  = id @ gi_z[t]                 (start; z negated for u=1-z)
    ps_u += W_z @ h_{t-1}   (stop)       (h is staged anyway; this keeps
                                          sigma(u) off the late mm phase)
    ps_n  = W_n @ nsm_{t-1}              (start)
    ps_r += W_r @ nsm_{t-1}              (leaves W_r loaded)
  on mm_{t-1} arrival:
    ps_r += W_r @ mm_{t-1}  (stop)       (NO weight reload)
    ps_n += W_n @ mm_{t-1}  (stop)       (one switch, gates m1)
    r    = sigmoid(ps_r)                 (ACT)
    u    = sigmoid(ps_u)                 (ACT)
    m1   = (ps_n + b_hh_n) * r           (DVE stt, fused bias)
    pre  = m1 + gi_n[t]                  (DVE)
    n    = tanh(pre)                     (ACT)
    hu   = u * h_{t-1}                   (DVE, under the tanh)
    nsm_t= h_{t-1} - hu                  (DVE, under the tanh)
    mm_t = u * n                         (DVE -> next tick's PE input)
    h_t  = mm_t + nsm_t                  (DVE, off-path, into staging)

  Variable length: gi_z carries -1e4 for t >= seq_len so u=0 and h
  freezes exactly (mm=0, nsm=h).

All PE operands fp16. gi = x @ w_ih.T (+ biases, + mask) precomputed
per 512-tick chunk from fp16 x via DMA-xbar transposes and [128,512]
matmuls, overlapped with the scan. Outputs staged fp16, PE-transposed
every 16 ticks (deferred one tick so the transpose's weight load
doesn't clobber the W_r preload), DMAed as fp32.
"""

import sys
import numpy as np

sys.path.insert(0, "/opt/trn_rl_repo")

B, T_FULL, I, H = 64, 2048, 128, 128
NCORES = 8
BC = B // NCORES          # sequences per core
BLK = 16                  # scan ticks per output block (BC*BLK = 128 cols)

_CACHE = {}

# Timing-bisect toggles (set by bench harnesses only; break correctness).
NOGI = False
NOOUT = False


def _build(T):
    from contextlib import ExitStack
    import concourse.bacc as bacc
    import concourse.mybir as mybir
    import concourse.tile as tile

    CHUNK = min(512, T)
    assert T % CHUNK == 0 and T % BLK == 0 and CHUNK % 16 == 0

    f32 = mybir.dt.float32
    f16 = mybir.dt.float16
    Alu = mybir.AluOpType
    Act = mybir.ActivationFunctionType

    nc = bacc.Bacc("TRN2", target_bir_lowering=False, debug=False,
                   num_devices=NCORES)

    xs = nc.dram_tensor("xs", [BC, T, I], f16, kind="ExternalInput").ap()
    wih = nc.dram_tensor("wih", [I, 3 * H], f16, kind="ExternalInput").ap()
    whh = nc.dram_tensor("whh", [H, 3 * H], f16, kind="ExternalInput").ap()
    bias3 = nc.dram_tensor("bias3", [H, 3], f32, kind="ExternalInput").ap()
    bhn1 = nc.dram_tensor("bhn1", [H, 1], f32, kind="ExternalInput").ap()
    ident = nc.dram_tensor("ident", [128, 128], f16, kind="ExternalInput").ap()
    mrow = nc.dram_tensor("mrow", [1, BC * T], f16, kind="ExternalInput").ap()
    ys = nc.dram_tensor("ys", [BC, T, H], f32, kind="ExternalOutput").ap()

    with tile.TileContext(nc) as tc, ExitStack() as ctx:
        const = ctx.enter_context(tc.tile_pool(name="const", bufs=1))
        xpool = ctx.enter_context(tc.tile_pool(name="x", bufs=3))
        gipool = ctx.enter_context(tc.tile_pool(name="gi", bufs=2))
        stpool = ctx.enter_context(tc.tile_pool(name="stage", bufs=3))
        opool = ctx.enter_context(tc.tile_pool(name="oblk", bufs=3))
        scratch = ctx.enter_context(tc.tile_pool(name="scr", bufs=2))
        hpool = ctx.enter_context(tc.tile_pool(name="h", bufs=3))
        ps_tr = ctx.enter_context(tc.tile_pool(name="ps_tr", bufs=1, space="PSUM"))
        ps_gi = ctx.enter_context(tc.tile_pool(name="ps_gi", bufs=1, space="PSUM"))
        ps_r = ctx.enter_context(tc.tile_pool(name="ps_r", bufs=2, space="PSUM"))
        ps_u = ctx.enter_context(tc.tile_pool(name="ps_u", bufs=2, space="PSUM"))
        ps_n = ctx.enter_context(tc.tile_pool(name="ps_n", bufs=2, space="PSUM"))

        wih_sb = const.tile([128, 3 * H], f16)
        nc.sync.dma_start(out=wih_sb[:], in_=wih)
        whh_sb = const.tile([128, 3 * H], f16)
        nc.sync.dma_start(out=whh_sb[:], in_=whh)
        bias_sb = const.tile([128, 3], f32)
        nc.sync.dma_start(out=bias_sb[:], in_=bias3)
        bhn_sb = const.tile([128, 1], f32)
        nc.sync.dma_start(out=bhn_sb[:], in_=bhn1)
        id_sb = const.tile([128, 128], f16)
        nc.sync.dma_start(out=id_sb[:], in_=ident)
        mrow_sb = const.tile([1, BC * T], f16)
        nc.sync.dma_start(out=mrow_sb[:], in_=mrow)
        ones_sb = const.tile([1, 128], f16)
        nc.vector.memset(ones_sb[:], 1.0)
        zmm = const.tile([128, BC], f16)
        nc.vector.memset(zmm[:], 0.0)
        znsm = const.tile([128, BC], f16)
        nc.vector.memset(znsm[:], 0.0)

        n_chunks = T // CHUNK

        def emit_gi(ck):
            """Precompute gi for chunk ck; returns (gir, giz, gin) views."""
            t0 = ck * CHUNK
            tiles = [gipool.tile([128, CHUNK * 8], f16, tag=f"gi{g}",
                                 name=f"gi{g}_c{ck}")
                     for g in range(3)]
            views = [tl.rearrange("p (t c) -> p t c", c=8) for tl in tiles]
            for b in range(BC):
                xt = xpool.tile([128, CHUNK], f16, tag="xT")
                nc.sync.dma_start_transpose(out=xt[:], in_=xs[b, t0:t0 + CHUNK, :])
                for g in range(3):
                    pg = ps_gi.tile([128, CHUNK], f32, tag="pgi")
                    nc.tensor.matmul(pg[:], wih_sb[:, g * 128:(g + 1) * 128],
                                     xt[:], start=True, stop=(g != 1))
                    if g == 1:  # z: add -1e4 mask via rank-1 matmul
                        nc.tensor.matmul(
                            pg[:], ones_sb[:],
                            mrow_sb[0:1, b * T + t0: b * T + t0 + CHUNK],
                            start=False, stop=True)
                    nc.scalar.activation(views[g][:, :, b], pg[:], Act.Identity,
                                         bias=bias_sb[:, g:g + 1])
            return views

        if NOGI:
            gfix = const.tile([128, 24], f16)
            nc.vector.memset(gfix[:], 0.1)
            gfix3 = gfix.rearrange("p (t c) -> p t c", c=8)

            class _V:
                def __init__(self, g):
                    self.g = g

                def __getitem__(self, idx):
                    return gfix3[:, self.g, :]

            gir3, giz3, gin3 = _V(0), _V(1), _V(2)
        else:
            gir3, giz3, gin3 = emit_gi(0)
        next_views = None
        mm_prev = zmm[:]
        nsm_prev = znsm[:]
        h_prev = None
        stage = None
        st3 = None
        pending = None  # (stage_tile, tb0) awaiting output flush

        for t in range(T):
            ck, t_rel = t // CHUNK, t % CHUNK
            tr = t % BLK
            if tr == 0:
                stage = stpool.tile([128, BC * BLK], f16, tag="st")
                st3 = stage.rearrange("p (b t) -> p b t", t=BLK)

            psr = ps_r.tile([128, 8], f32, tag="r")
            psu = ps_u.tile([128, 8], f32, tag="u")
            psn = ps_n.tile([128, 8], f32, tag="n")
            h_in = znsm[:] if h_prev is None else h_prev
            # -- pre-mm phase: early operands, all weight switches here.
            # The z gate consumes the staged h directly (one matmul, ready
            # just after mm), so sigma(u) never waits on a late mm-phase
            # weight switch. --
            nc.tensor.matmul(psr[:], id_sb[:], gir3[:, t_rel, :],
                             start=True, stop=False, skip_group_check=True)
            nc.tensor.matmul(psu[:], id_sb[:], giz3[:, t_rel, :],
                             start=True, stop=False, skip_group_check=True)
            nc.tensor.matmul(psu[:], whh_sb[:, 128:256], h_in,
                             start=False, stop=True, skip_group_check=True)
            nc.tensor.matmul(psn[:], whh_sb[:, 256:384], nsm_prev,
                             start=True, stop=False, skip_group_check=True)
            nc.tensor.matmul(psr[:], whh_sb[:, 0:128], nsm_prev,
                             start=False, stop=False, skip_group_check=True)
            # -- mm phase: first MM reuses the W_r stationary --
            nc.tensor.matmul(psr[:], whh_sb[:, 0:128], mm_prev,
                             start=False, stop=True, skip_group_check=True)
            mm_z = nc.tensor.matmul(psn[:], whh_sb[:, 256:384], mm_prev,
                                    start=False, stop=True,
                                    skip_group_check=True)

            rz = scratch.tile([128, 16], f32, tag="rz")
            nc.scalar.activation(rz[:, 0:8], psr[:], Act.Sigmoid)
            nc.scalar.activation(rz[:, 8:16], psu[:], Act.Sigmoid)
            m1 = scratch.tile([128, 8], f32, tag="m1")
            nc.vector.scalar_tensor_tensor(out=m1[:], in0=psn[:],
                                           scalar=bhn_sb[:, 0:1],
                                           in1=rz[:, 0:8],
                                           op0=Alu.add, op1=Alu.mult)
            pre = scratch.tile([128, 8], f32, tag="pre")
            pre_inst = nc.vector.tensor_add(out=pre[:], in0=m1[:],
                                            in1=gin3[:, t_rel, :])
            nt = scratch.tile([128, 8], f32, tag="nt")
            tanh_inst = nc.scalar.activation(nt[:], pre[:], Act.Tanh)
            # hu/nsm run under the tanh; keep them off the m1->pre chain.
            nsm_t = hpool.tile([128, 8], f16, tag="nsm")
            if h_prev is None:
                nc.vector.memset(nsm_t[:], 0.0)
            else:
                hu = scratch.tile([128, 8], f32, tag="hu")
                hu_inst = nc.vector.tensor_mul(out=hu[:], in0=rz[:, 8:16],
                                               in1=h_prev)
                tile.add_dep_helper(hu_inst.ins, pre_inst.ins, sync=False,
                                    reason="keep hu off the m1->pre chain")
                nc.vector.tensor_sub(out=nsm_t[:], in0=h_prev, in1=hu[:])
            mm_t = hpool.tile([128, 8], f16, tag="mm")
            nc.vector.tensor_mul(out=mm_t[:], in0=rz[:, 8:16], in1=nt[:])
            hslot = st3[:, :, tr]
            nc.vector.tensor_add(out=hslot, in0=mm_t[:], in1=nsm_t[:])
            h_prev = hslot
            mm_prev = mm_t[:]
            nsm_prev = nsm_t[:]

            # -- deferred output flush: one tick after the block completes,
            # ordered after this tick's last mm-matmul so the transpose's
            # weight load never clobbers the W_r preload mid-chain --
            if pending is not None and not NOOUT:
                pstage, tb0 = pending
                ptr = ps_tr.tile([128, 128], f16, tag="tr")
                tr_inst = nc.tensor.transpose(ptr[:], pstage[:], id_sb[:])
                tile.add_dep_helper(tr_inst.ins, mm_z.ins, sync=False,
                                    reason="flush transpose after mm phase")
                ob = opool.tile([128, 128], f32, tag="ob")
                ob_inst = nc.scalar.copy(ob[:], ptr[:])
                tile.add_dep_helper(ob_inst.ins, tanh_inst.ins, sync=False,
                                    reason="ob copy after tanh")
                for b in range(BC):
                    nc.sync.dma_start(out=ys[b, tb0:tb0 + BLK, :],
                                      in_=ob[b * BLK:(b + 1) * BLK, :])
                pending = None
            if tr == BLK - 1:
                pending = (stage, t - BLK + 1)

            if not NOGI:
                if t_rel == BLK - 1 and ck + 1 < n_chunks:
                    next_views = emit_gi(ck + 1)
                if t_rel == CHUNK - 1 and ck + 1 < n_chunks:
                    gir3, giz3, gin3 = next_views

        # final block flush
        if pending is not None and not NOOUT:
            pstage, tb0 = pending
            ptr = ps_tr.tile([128, 128], f16, tag="tr")
            nc.tensor.transpose(ptr[:], pstage[:], id_sb[:])
            ob = opool.tile([128, 128], f32, tag="ob")
            nc.scalar.copy(ob[:], ptr[:])
            for b in range(BC):
                nc.sync.dma_start(out=ys[b, tb0:tb0 + BLK, :],
                                  in_=ob[b * BLK:(b + 1) * BLK, :])

    nc.compile()
    return nc


def _host_prep(x, seq_len, w_ih, w_hh, b_ih, b_hh):
    T = x.shape[1]
    wihT = w_ih.T.astype(np.float32).copy()
    whhT = w_hh.T.astype(np.float32).copy()
    wihT[:, 128:256] *= -1.0
    whhT[:, 128:256] *= -1.0
    bias3 = np.stack([
        b_ih[0:128] + b_hh[0:128],
        -(b_ih[128:256] + b_hh[128:256]),
        b_ih[256:384],
    ], axis=1).astype(np.float32)
    bhn1 = b_hh[256:384].astype(np.float32)[:, None].copy()
    identity = np.eye(128, dtype=np.float16)
    wih16 = np.ascontiguousarray(wihT.astype(np.float16))
    whh16 = np.ascontiguousarray(whhT.astype(np.float16))
    x16 = x.astype(np.float16)
    in_maps = []
    for c in range(NCORES):
        sl = seq_len[c * BC:(c + 1) * BC].astype(np.int64)
        mask = (np.arange(T)[None, :] >= sl[:, None]).astype(np.float16)
        mrow_np = (-1e4 * mask).reshape(1, BC * T).astype(np.float16)
        in_maps.append({
            "xs": np.ascontiguousarray(x16[c * BC:(c + 1) * BC]),
            "wih": wih16, "whh": whh16,
            "bias3": bias3, "bhn1": bhn1,
            "ident": identity, "mrow": mrow_np,
        })
    return in_maps


LAST_RESULTS = None


def kernel(x, seq_len, w_ih, w_hh, b_ih, b_hh):
    global LAST_RESULTS
    from concourse import bass_utils
    T = x.shape[1]
    if T not in _CACHE:
        _CACHE[T] = _build(T)
    nc = _CACHE[T]
    in_maps = _host_prep(np.asarray(x), np.asarray(seq_len), np.asarray(w_ih),
                         np.asarray(w_hh), np.asarray(b_ih), np.asarray(b_hh))
    res = bass_utils.run_bass_kernel_spmd(nc, in_maps,
                                          core_ids=list(range(NCORES)))
    LAST_RESULTS = res
    y = np.concatenate([res.results[c]["ys"] for c in range(NCORES)], axis=0)
    return y.astype(np.float32)
